# revision 22
# baseline (speedup 1.0000x reference)
"""DSS attention Trainium2 kernel (8 NeuronCores, row-sharded).

Reference math (B=1, N=4096, C=512, H=8, D=64, R=32, BLK=16):
  q = (x @ q_w1.T) @ q_w2.T ; kv = (x @ kv_w1.T) @ kv_w2.T ; split k, v per head
  s = (q*sqrt(D)) @ k.T ; attn = softmax(s) * blockdiag_causal_mask(16)
  wv = attn @ v ; dyn = (wv*dw_w+dw_b) @ pw_w.T + pw_b ; y = ((dyn+x) @ p_w1.T) @ p_w2.T

Key structure: the mask is applied AFTER the full-row softmax, so
  wv_i = (sum_{j in blk(i), j<=i} e^{s_ij} v_j) / (sum_{all j} e^{s_ij}).
Only the denominator is O(N^2): bf16 score matmuls into PSUM, ACT Exp with
fused accum_out row-sums straight off PSUM (ACT is the bottleneck engine at
~1.2GHz x 128 lanes over N^2/8 elements per core). The numerator only touches
the 16-wide diagonal blocks, computed transposed so wv lands [i, d] for a
per-partition 1/d scale, then PE-transposed into [c, i] for the epilogue.

The epilogue is split so precision costs nothing:
  y = [(wv*dw_w) @ pw^T  +  (dw_b @ pw^T + pw_b + x)] @ p1^T @ p2^T
The attention part (left) is tiny (dw_w ~ 0.02) and runs bf16 at the tail;
the x part (right) is fp32 and runs mid-stream under the ACT shadow.

Sharding: each core takes 512 query rows x all 8 heads. Per-core x arrives
column-rolled so the core's rows come first -> one SPMD program, static
offsets. Heads are processed in pairs so projections/copies use all 128
partitions. PSUM: psDen ([128,1536] x2 = 6 banks) for the score/exp stream +
one shared 2-slot 1-bank-tile pool (psS) for everything else. Emission order
is chosen so psS allocation order matches execution order: pair p+1's
projections are emitted BEFORE pair p's denominator/numerator stream.
"""

import sys

sys.path.insert(0, "/opt/trn_rl_repo")

import numpy as np
import ml_dtypes

import concourse.bass as bass
import concourse.tile as tile
from concourse import bacc, mybir
from concourse.bass_utils import run_bass_kernel_spmd

N, C, H, D, R, BLK = 4096, 512, 8, 64, 32, 16
NCORES = 8
RPC = N // NCORES          # rows per core = 512
IC = RPC // 128            # i-chunks per core = 4
SCALE = float(np.sqrt(D))
DEN_PARTS = [(0, 1536), (1536, 1536), (3072, 1024)]   # j-splits per (h, ic)

F32 = mybir.dt.float32
BF16 = mybir.dt.bfloat16
FP8 = mybir.dt.float8e4
AF = mybir.ActivationFunctionType
OP = mybir.AluOpType
bf16 = ml_dtypes.bfloat16

_CACHE = {}


def _build_program():
    nc = bacc.Bacc("TRN2", target_bir_lowering=False, debug=False,
                   num_devices=NCORES)

    def din(name, shape, dt):
        return nc.dram_tensor(name, shape, dt, kind="ExternalInput").ap()

    xt_d = din("xt", [C, N], BF16)            # x^T, columns rolled: core rows first
    xres_d = din("xres", [C, RPC], F32)       # fp32 x^T slice of core rows
    qw1t_d = din("qw1t", [C, R], BF16)
    kvw1t_d = din("kvw1t", [C, 2 * R], BF16)
    wq_d = din("wq", [R, C], BF16)            # head h cols h*64:+64 (SCALE folded)
    wk_d = din("wk", [2 * R, C], BF16)
    wv_d = din("wv", [2 * R, C], BF16)
    pwtb_d = din("pwtb", [C, C], BF16)        # pw_w.T (bf16)
    pw1t_d = din("pw1t", [C, R], F32)         # p_w1.T fp32 (x path)
    pw2t_d = din("pw2t", [R, C], F32)         # p_w2.T fp32 (x path)
    pw1tb_d = din("pw1tb", [C, R], BF16)      # p_w1.T bf16 (attention path)
    pw2tb_d = din("pw2tb", [R, C], BF16)      # p_w2.T bf16 (attention path)
    dwc_d = din("dwc", [128, 4], F32)
    cvc_d = din("cvc", [128, 4], F32)         # dw_b @ pw_w.T + pw_b
    maskr_d = din("maskr", [128, 1024], BF16)  # maskT tiled 8x along free
    ident_d = din("ident", [128, 128], F32)

    yt_d = nc.dram_tensor("yt", [C, RPC], F32, kind="ExternalOutput").ap()

    with tile.TileContext(nc) as tc:
        with (
            tc.tile_pool(name="consts", bufs=1) as consts,
            tc.tile_pool(name="persist", bufs=1) as persist,
            tc.tile_pool(name="work", bufs=3) as work,
            tc.tile_pool(name="psS", bufs=2, space="PSUM") as psS,
            tc.tile_pool(name="psDen", bufs=2, space="PSUM") as psDen,
        ):
            # ---- loads ----
            # x streams on the HWDGE (sync) queue in j-quarters; everything
            # else rides the gpsimd SWDGE queue in parallel, one DMA per
            # tensor (3D strided APs), projection weights first.
            # x streams on the sync/HWDGE queue immediately, in eighths so
            # downstream compute starts as soon as the first slab lands;
            # everything else rides the gpsimd SWDGE queue in parallel
            xt = consts.tile([128, 4, N], BF16)
            xt_r = xt_d.rearrange("(c p) j -> p c j", p=128)
            for jq in range(8):
                js = slice(jq * 512, (jq + 1) * 512)
                nc.sync.dma_start(out=xt[:, :, js], in_=xt_r[:, :, js])

            qw1t = consts.tile([128, 4, R], BF16)
            nc.gpsimd.dma_start(out=qw1t,
                                in_=qw1t_d.rearrange("(c p) r -> p c r", p=128))
            kvw1t = consts.tile([128, 4, 2 * R], BF16)
            nc.gpsimd.dma_start(out=kvw1t,
                                in_=kvw1t_d.rearrange("(c p) r -> p c r", p=128))
            wq = consts.tile([R, C], BF16)
            nc.gpsimd.dma_start(out=wq, in_=wq_d)
            wk = consts.tile([2 * R, C], BF16)
            nc.gpsimd.dma_start(out=wk, in_=wk_d)
            wv = consts.tile([2 * R, C], BF16)
            nc.gpsimd.dma_start(out=wv, in_=wv_d)
            maskr = consts.tile([128, 1024], BF16)
            nc.gpsimd.dma_start(out=maskr, in_=maskr_d)
            ident = consts.tile([128, 128], F32)
            nc.gpsimd.dma_start(out=ident, in_=ident_d)
            cvc = consts.tile([128, 4], F32)
            nc.gpsimd.dma_start(out=cvc, in_=cvc_d)
            dwc = consts.tile([128, 4], F32)
            nc.gpsimd.dma_start(out=dwc, in_=dwc_d)
            xres = consts.tile([128, 4, RPC], F32)
            pwtb = consts.tile([128, 4, C], BF16)
            pw1t = consts.tile([128, 4, R], F32)
            pw1tb = consts.tile([128, 4, R], BF16)
            pw2t = consts.tile([R, C], F32)
            pw2tb = consts.tile([R, C], BF16)

            # persistent intermediates
            xrt = persist.tile([R, RPC], BF16)             # xr^T (core rows)
            xkvt = persist.tile([2 * R, N], BF16)          # xkv^T (all rows)
            qt2 = persist.tile([128, 4, RPC], BF16)        # Q^T head pairs
            kt2 = persist.tile([128, 4, N], BF16)          # K^T head pairs
            v2 = persist.tile([128, IC, 4, 128], BF16)     # V rows, head pairs
            et = persist.tile([128, 4096], BF16)           # masked diag-block exp
            dsums = persist.tile([128, H * IC * 3], F32)   # exp row-sum parts
            dsums0 = persist.tile([128, 5], F32)           # first group: 4 parts + total
            dtot = persist.tile([128, H * IC * 2], F32)
            recips = persist.tile([128, H * IC], F32)
            wvt = persist.tile([128, 4, RPC], F32)         # wv^T assembled [c, i]
            dyn0b = persist.tile([128, 4, RPC], BF16)      # (wv*dw)^T bf16
            ya = persist.tile([128, 4, RPC], F32)          # fp32 x-path output
            y1a = persist.tile([128, 4, RPC], F32)
            y1b = persist.tile([128, 4, RPC], BF16)
            t2a = persist.tile([R, RPC], F32)
            t2b = persist.tile([R, RPC], BF16)
            scratch = persist.tile([128, 1536], FP8)       # ACT exp discard target

            def sps(shape):
                return psS.tile(shape, F32, tag="s", name="s_tile")

            def project_pair(p, jcs=range(8), head=True):
                """Q^T/K^T/V for heads (2p, 2p+1), stacked on partitions."""
                psl = slice(p * 128, (p + 1) * 128)
                if head:
                    ps = sps([128, 512])
                    nc.tensor.matmul(ps, wq[:, psl], xrt, start=True, stop=True)
                    nc.vector.tensor_copy(qt2[:, p, :], ps)
                for jc in jcs:
                    js = slice(jc * 512, (jc + 1) * 512)
                    ps = sps([128, 512])
                    nc.tensor.matmul(ps, wk[:, psl], xkvt[:, js],
                                     start=True, stop=True)
                    nc.vector.tensor_copy(kt2[:, p, js], ps)
                if head:
                    for ic in range(IC):
                        cs = slice(ic * 128, (ic + 1) * 128)
                        ps = sps([128, 128])
                        nc.tensor.matmul(ps, xkvt[:, cs], wv[:, psl],
                                         start=True, stop=True)
                        nc.vector.tensor_copy(v2[:, ic, p, :], ps)

            def diag_pair(p):
                """Diagonal-block S^T -> exp -> mask, batched for the pair."""
                psd = psDen.tile([128, 1536], F32, tag="big")
                for t in range(8):
                    hh, ic = divmod(t, IC)
                    poff = hh * 64
                    cs = slice(ic * 128, (ic + 1) * 128)
                    nc.tensor.matmul(psd[:, t * 128:(t + 1) * 128],
                                     kt2[poff:poff + 64, p, cs],
                                     qt2[poff:poff + 64, p, cs],
                                     start=True, stop=True)
                esl = slice(p * 1024, (p + 1) * 1024)
                nc.scalar.activation(et[:, esl], psd[:, 0:1024], AF.Exp)
                nc.vector.tensor_mul(et[:, esl], et[:, esl], maskr)

            anchors = {}

            def den_num_pair(p):
                """Denominator exp+rowsum stream and per-(h,ic) numerators."""
                for hh in range(2):
                    h = 2 * p + hh
                    poff = hh * 64
                    for ic in range(IC):
                        if p == 0 and hh == 0 and ic == 0:
                            continue   # handled in the interleaved prologue
                        cs = slice(ic * 128, (ic + 1) * 128)
                        ki = h * IC + ic
                        k0 = ki * 3
                        for kpart, (off, ln) in enumerate(DEN_PARTS):
                            psq = psDen.tile([128, 1536], F32, tag="big")
                            for m in range(ln // 512):
                                nc.tensor.matmul(
                                    psq[:, m * 512:(m + 1) * 512],
                                    qt2[poff:poff + 64, p, cs],
                                    kt2[poff:poff + 64, p,
                                        off + m * 512:off + (m + 1) * 512],
                                    start=True, stop=True)
                            ai = nc.scalar.activation(
                                scratch[:, 0:ln], psq[:, 0:ln], AF.Exp,
                                accum_out=dsums[:, k0 + kpart:k0 + kpart + 1])
                            anchors.setdefault(p, []).append(ai)
                        nc.vector.tensor_add(dtot[:, 2 * ki:2 * ki + 1],
                                             dsums[:, k0:k0 + 1],
                                             dsums[:, k0 + 1:k0 + 2])
                        nc.vector.tensor_add(dtot[:, 2 * ki + 1:2 * ki + 2],
                                             dtot[:, 2 * ki:2 * ki + 1],
                                             dsums[:, k0 + 2:k0 + 3])
                        nc.vector.reciprocal(recips[:, ki:ki + 1],
                                             dtot[:, 2 * ki + 1:2 * ki + 2])
                        # numerator for this (h, ic)
                        tblk = (p * 8 + hh * IC + ic) * 128
                        psw = sps([128, D])
                        nc.tensor.matmul(psw, et[:, tblk:tblk + 128],
                                         v2[:, ic, p, poff:poff + 64],
                                         start=True, stop=True)
                        wvs = work.tile([128, D], F32, tag="wvs")
                        nc.vector.tensor_scalar_mul(wvs, psw,
                                                    recips[:, ki:ki + 1])
                        pst = sps([D, 128])
                        nc.tensor.transpose(pst, wvs, ident)
                        nc.vector.tensor_copy(
                            wvt[poff:poff + 64, p, ic * 128:(ic + 1) * 128], pst)
                # attention-path depthwise scale (dw_b folded into cvec)
                nc.vector.tensor_scalar_mul(wvt[:, p, :], wvt[:, p, :],
                                            dwc[:, p:p + 1])
                nc.vector.tensor_copy(dyn0b[:, p, :], wvt[:, p, :])

            def late_loads():
                """Epilogue-only tensors: DMA them only after pair-0's first
                exps so they don't steal DMA-bus bandwidth from xt."""
                from concourse.bass import _add_dep_helper
                a = anchors[0][2].ins
                for dmi in (
                    nc.gpsimd.dma_start(
                        out=xres,
                        in_=xres_d.rearrange("(c p) j -> p c j", p=128)),
                    nc.gpsimd.dma_start(
                        out=pwtb,
                        in_=pwtb_d.rearrange("(c p) r -> p c r", p=128)),
                    nc.gpsimd.dma_start(
                        out=pw1t,
                        in_=pw1t_d.rearrange("(c p) r -> p c r", p=128)),
                    nc.gpsimd.dma_start(
                        out=pw1tb,
                        in_=pw1tb_d.rearrange("(c p) r -> p c r", p=128)),
                    nc.gpsimd.dma_start(out=pw2t, in_=pw2t_d),
                    nc.gpsimd.dma_start(out=pw2tb, in_=pw2tb_d),
                ):
                    _add_dep_helper(dmi.ins, a, True, "defer epilogue DMAs")

            def xpath():
                """fp32 x-path: ya = (x + cvec) @ p1^T @ p2^T (runs under the
                ACT shadow mid-stream)."""
                from concourse.bass import _add_dep_helper
                for t in range(4):
                    ai = nc.vector.tensor_scalar_add(y1a[:, t, :],
                                                     xres[:, t, :],
                                                     cvc[:, t:t + 1])
                    _add_dep_helper(ai.ins, anchors[1][6 * t].ins, False,
                                    "spread xpath adds across pair 1")
                psa = sps([R, 512])
                for c in range(4):
                    mi = nc.tensor.matmul(psa, pw1t[:, c, :], y1a[:, c, :],
                                          start=(c == 0), stop=(c == 3))
                    _add_dep_helper(mi.ins, anchors[2][2 + 4 * c].ins, False,
                                    "spread xpath p1 across pair 2")
                nc.vector.tensor_copy(t2a, psa)
                for t in range(4):
                    pya = sps([128, 512])
                    mi = nc.tensor.matmul(pya, pw2t[:, t * 128:(t + 1) * 128],
                                          t2a, start=True, stop=True)
                    _add_dep_helper(mi.ins, anchors[2][18 + t].ins, False,
                                    "spread xpath p2 across pair 2")
                    nc.vector.tensor_copy(ya[:, t, :], pya)

            def xkvt_chunk(jc):
                js = slice(jc * 512, (jc + 1) * 512)
                ps = sps([2 * R, 512])
                for c in range(4):
                    nc.tensor.matmul(ps, kvw1t[:, c, :], xt[:, c, js],
                                     start=(c == 0), stop=(c == 3))
                nc.vector.tensor_copy(xkvt[:, js], ps)

            # ---- startup: emit only what the first score scans need, so
            # PE's in-order stream is not stuck behind late xt slabs ----
            ps = sps([R, 512])
            for c in range(4):
                nc.tensor.matmul(ps, qw1t[:, c, :], xt[:, c, 0:RPC],
                                 start=(c == 0), stop=(c == 3))
            nc.vector.tensor_copy(xrt, ps)
            for jc in range(3):
                xkvt_chunk(jc)
            project_pair(0, jcs=range(3))
            diag_pair(0)

            # first (h0, ic0) group: parts interleaved with the remaining
            # xkvt/kt chunk projections in j order
            poff0 = 0
            cs0 = slice(0, 128)
            parts0 = [(0, 512), (512, 1024), (1536, 1536), (3072, 1024)]
            for kpart, (off, ln) in enumerate(parts0):
                psq = psDen.tile([128, 1536], F32, tag="big")
                for m in range(ln // 512):
                    nc.tensor.matmul(
                        psq[:, m * 512:(m + 1) * 512],
                        qt2[poff0:poff0 + 64, 0, cs0],
                        kt2[poff0:poff0 + 64, 0,
                            off + m * 512:off + (m + 1) * 512],
                        start=True, stop=True)
                ai = nc.scalar.activation(scratch[:, 0:ln], psq[:, 0:ln],
                                          AF.Exp,
                                          accum_out=dsums0[:, kpart:kpart + 1])
                anchors.setdefault(0, []).append(ai)
                if kpart == 0:
                    xkvt_chunk(3)
                    project_pair(0, jcs=[3], head=False)
                elif kpart == 1:
                    for jc in (4, 5):
                        xkvt_chunk(jc)
                    project_pair(0, jcs=[4, 5], head=False)
                elif kpart == 2:
                    for jc in (6, 7):
                        xkvt_chunk(jc)
                    project_pair(0, jcs=[6, 7], head=False)
            nc.vector.tensor_add(dtot[:, 0:1], dsums0[:, 0:1], dsums0[:, 1:2])
            nc.vector.tensor_add(dtot[:, 1:2], dsums0[:, 2:3], dsums0[:, 3:4])
            nc.vector.tensor_add(dsums0[:, 4:5], dtot[:, 0:1], dtot[:, 1:2])
            nc.vector.reciprocal(recips[:, 0:1], dsums0[:, 4:5])
            psw = sps([128, D])
            nc.tensor.matmul(psw, et[:, 0:128], v2[:, 0, 0, 0:64],
                             start=True, stop=True)
            wvs = work.tile([128, D], F32, tag="wvs")
            nc.vector.tensor_scalar_mul(wvs, psw, recips[:, 0:1])
            pst = sps([D, 128])
            nc.tensor.transpose(pst, wvs, ident)
            nc.vector.tensor_copy(wvt[0:64, 0, 0:128], pst)

            for p in range(4):
                if p < 4 - 1:
                    project_pair(p + 1)   # executes under pair p's ACT shadow
                den_num_pair(p)
                if p == 0:
                    late_loads()
                if p == 2:
                    xpath()   # fp32 x-path, runs under the ACT shadow
                if p < 4 - 1:
                    diag_pair(p + 1)

            # ---- bf16 attention-path epilogue + combine ----
            for tp in range(4):
                pse = psDen.tile([128, 1536], F32, tag="big", name="pse")
                for c in range(4):
                    nc.tensor.matmul(pse[:, 0:512],
                                     pwtb[:, c, tp * 128:(tp + 1) * 128],
                                     dyn0b[:, c, :], start=(c == 0), stop=(c == 3))
                nc.vector.tensor_copy(y1b[:, tp, :], pse[:, 0:512])
            psb = sps([R, 512])
            for c in range(4):
                nc.tensor.matmul(psb, pw1tb[:, c, :], y1b[:, c, :],
                                 start=(c == 0), stop=(c == 3))
            nc.vector.tensor_copy(t2b, psb)
            for t in range(4):
                psy = psDen.tile([128, 1536], F32, tag="big", name="psy")
                nc.tensor.matmul(psy[:, 0:512], pw2tb[:, t * 128:(t + 1) * 128],
                                 t2b, start=True, stop=True)
                ysb = work.tile([128, RPC], F32, tag="ysb")
                nc.vector.tensor_add(ysb, psy[:, 0:512], ya[:, t, :])
                eng = nc.sync if t % 2 == 0 else nc.gpsimd
                eng.dma_start(out=yt_d[t * 128:(t + 1) * 128, :], in_=ysb)

    nc.compile()
    return nc


def _prep_inputs(inputs):
    x = np.asarray(inputs["x"], np.float32)[0]        # [N, C]
    q_w1 = np.asarray(inputs["q_w1"], np.float32)
    q_w2 = np.asarray(inputs["q_w2"], np.float32)
    kv_w1 = np.asarray(inputs["kv_w1"], np.float32)
    kv_w2 = np.asarray(inputs["kv_w2"], np.float32)
    dw_w = np.asarray(inputs["dw_w"], np.float32)
    dw_b = np.asarray(inputs["dw_b"], np.float32)
    pw_w = np.asarray(inputs["pw_w"], np.float32)
    pw_b = np.asarray(inputs["pw_b"], np.float32)
    p_w1 = np.asarray(inputs["p_w1"], np.float32)
    p_w2 = np.asarray(inputs["p_w2"], np.float32)

    xT = np.ascontiguousarray(x.T)                    # [C, N]
    xT_bf = xT.astype(bf16)

    wq = np.empty((R, C), np.float32)
    wkm = np.empty((2 * R, C), np.float32)
    wvm = np.empty((2 * R, C), np.float32)
    for h in range(H):
        hs = slice(h * D, (h + 1) * D)
        wq[:, hs] = q_w2[hs, :].T * SCALE
        wkm[:, hs] = kv_w2[hs, :].T
        wvm[:, hs] = kv_w2[C + h * D:C + (h + 1) * D, :].T

    jj, ii = np.meshgrid(np.arange(128), np.arange(128), indexing="ij")
    maskt = (((ii // BLK) == (jj // BLK)) & (ii >= jj)).astype(bf16)
    maskr = np.tile(maskt, (1, 8))
    cvec = dw_b @ pw_w.T + pw_b

    shared = {
        "qw1t": np.ascontiguousarray(q_w1.T).astype(bf16),
        "kvw1t": np.ascontiguousarray(kv_w1.T).astype(bf16),
        "wq": wq.astype(bf16),
        "wk": wkm.astype(bf16),
        "wv": wvm.astype(bf16),
        "pwtb": np.ascontiguousarray(pw_w.T).astype(bf16),
        "pw1t": np.ascontiguousarray(p_w1.T),
        "pw2t": np.ascontiguousarray(p_w2.T),
        "pw1tb": np.ascontiguousarray(p_w1.T).astype(bf16),
        "pw2tb": np.ascontiguousarray(p_w2.T).astype(bf16),
        "dwc": np.ascontiguousarray(dw_w.reshape(4, 128).T),
        "cvc": np.ascontiguousarray(cvec.reshape(4, 128).T),
        "maskr": np.ascontiguousarray(maskr),
        "ident": np.eye(128, dtype=np.float32),
    }
    in_maps = []
    for core in range(NCORES):
        r0 = core * RPC
        rolled = np.concatenate([xT_bf[:, r0:], xT_bf[:, :r0]], axis=1)
        m = dict(shared)
        m["xt"] = np.ascontiguousarray(rolled)
        m["xres"] = np.ascontiguousarray(xT[:, r0:r0 + RPC])
        in_maps.append(m)
    return in_maps


def kernel(**inputs):
    if "nc" not in _CACHE:
        _CACHE["nc"] = _build_program()
    nc = _CACHE["nc"]
    in_maps = _prep_inputs(inputs)
    res = run_bass_kernel_spmd(nc, in_maps, core_ids=list(range(NCORES)))
    y = np.empty((N, C), np.float32)
    for core in range(NCORES):
        r0 = core * RPC
        y[r0:r0 + RPC, :] = res.results[core]["yt"].T
    return y.reshape(1, N, C)


# revision 25
# speedup vs baseline: 1.0319x; 1.0319x over previous
"""DSS attention Trainium2 kernel (8 NeuronCores, row-sharded).

Reference math (B=1, N=4096, C=512, H=8, D=64, R=32, BLK=16):
  q = (x @ q_w1.T) @ q_w2.T ; kv = (x @ kv_w1.T) @ kv_w2.T ; split k, v per head
  s = (q*sqrt(D)) @ k.T ; attn = softmax(s) * blockdiag_causal_mask(16)
  wv = attn @ v ; dyn = (wv*dw_w+dw_b) @ pw_w.T + pw_b ; y = ((dyn+x) @ p_w1.T) @ p_w2.T

Key structure: the mask is applied AFTER the full-row softmax, so
  wv_i = (sum_{j in blk(i), j<=i} e^{s_ij} v_j) / (sum_{all j} e^{s_ij}).
Only the denominator is O(N^2): bf16 score matmuls into PSUM, ACT Exp with
fused accum_out row-sums straight off PSUM (ACT is the bottleneck engine at
~1.2GHz x 128 lanes over N^2/8 elements per core). The numerator only touches
the 16-wide diagonal blocks, computed transposed so wv lands [i, d] for a
per-partition 1/d scale, then PE-transposed into [c, i] for the epilogue.

The epilogue is split so precision costs nothing:
  y = [(wv*dw_w) @ pw^T  +  (dw_b @ pw^T + pw_b + x)] @ p1^T @ p2^T
The attention part (left) is tiny (dw_w ~ 0.02) and runs bf16 at the tail;
the x part (right) is fp32 and runs mid-stream under the ACT shadow.

Sharding: each core takes 512 query rows x all 8 heads. Per-core x arrives
column-rolled so the core's rows come first -> one SPMD program, static
offsets. Heads are processed in pairs so projections/copies use all 128
partitions. PSUM: psDen ([128,1536] x2 = 6 banks) for the score/exp stream +
one shared 2-slot 1-bank-tile pool (psS) for everything else. Emission order
is chosen so psS allocation order matches execution order: pair p+1's
projections are emitted BEFORE pair p's denominator/numerator stream.
"""

import sys

sys.path.insert(0, "/opt/trn_rl_repo")

import numpy as np
import ml_dtypes

import concourse.bass as bass
import concourse.tile as tile
from concourse import bacc, mybir
from concourse.bass_utils import run_bass_kernel_spmd

N, C, H, D, R, BLK = 4096, 512, 8, 64, 32, 16
NCORES = 8
RPC = N // NCORES          # rows per core = 512
IC = RPC // 128            # i-chunks per core = 4
SCALE = float(np.sqrt(D))
DEN_PARTS = [(0, 1536), (1536, 1536), (3072, 1024)]   # j-splits per (h, ic)

F32 = mybir.dt.float32
BF16 = mybir.dt.bfloat16
FP8 = mybir.dt.float8e4
AF = mybir.ActivationFunctionType
OP = mybir.AluOpType
bf16 = ml_dtypes.bfloat16

_CACHE = {}


def _build_program():
    nc = bacc.Bacc("TRN2", target_bir_lowering=False, debug=False,
                   num_devices=NCORES)

    def din(name, shape, dt):
        return nc.dram_tensor(name, shape, dt, kind="ExternalInput").ap()

    xt_d = din("xt", [C, N], BF16)            # x^T, columns rolled: core rows first
    xres_d = din("xres", [C, RPC], F32)       # fp32 x^T slice of core rows
    qw1t_d = din("qw1t", [C, R], BF16)
    kvw1t_d = din("kvw1t", [C, 2 * R], BF16)
    wq_d = din("wq", [R, C], BF16)            # head h cols h*64:+64 (SCALE folded)
    wk_d = din("wk", [2 * R, C], BF16)
    wv_d = din("wv", [2 * R, C], BF16)
    pwtb_d = din("pwtb", [C, C], BF16)        # pw_w.T (bf16)
    pw1t_d = din("pw1t", [C, R], F32)         # p_w1.T fp32 (x path)
    pw2t_d = din("pw2t", [R, C], F32)         # p_w2.T fp32 (x path)
    pw1tb_d = din("pw1tb", [C, R], BF16)      # p_w1.T bf16 (attention path)
    pw2tb_d = din("pw2tb", [R, C], BF16)      # p_w2.T bf16 (attention path)
    dwc_d = din("dwc", [128, 4], F32)
    cvc_d = din("cvc", [128, 4], F32)         # dw_b @ pw_w.T + pw_b
    maskr_d = din("maskr", [128, 1024], BF16)  # maskT tiled 8x along free
    ident_d = din("ident", [128, 128], F32)
    identb_d = din("identb", [128, 128], BF16)

    yt_d = nc.dram_tensor("yt", [C, RPC], F32, kind="ExternalOutput").ap()

    with tile.TileContext(nc) as tc:
        with (
            tc.tile_pool(name="consts", bufs=1) as consts,
            tc.tile_pool(name="persist", bufs=1) as persist,
            tc.tile_pool(name="work", bufs=3) as work,
            tc.tile_pool(name="psS", bufs=2, space="PSUM") as psS,
            tc.tile_pool(name="psDen", bufs=2, space="PSUM") as psDen,
        ):
            # ---- loads ----
            # x streams on the HWDGE (sync) queue in j-quarters; everything
            # else rides the gpsimd SWDGE queue in parallel, one DMA per
            # tensor (3D strided APs), projection weights first.
            # x streams on the sync/HWDGE queue immediately, in eighths so
            # downstream compute starts as soon as the first slab lands;
            # everything else rides the gpsimd SWDGE queue in parallel
            xt = consts.tile([128, 4, N], BF16)
            xt_r = xt_d.rearrange("(c p) j -> p c j", p=128)
            for jq in range(8):
                js = slice(jq * 512, (jq + 1) * 512)
                nc.sync.dma_start(out=xt[:, :, js], in_=xt_r[:, :, js])

            qw1t = consts.tile([128, 4, R], BF16)
            nc.gpsimd.dma_start(out=qw1t,
                                in_=qw1t_d.rearrange("(c p) r -> p c r", p=128))
            kvw1t = consts.tile([128, 4, 2 * R], BF16)
            nc.gpsimd.dma_start(out=kvw1t,
                                in_=kvw1t_d.rearrange("(c p) r -> p c r", p=128))
            wq = consts.tile([R, C], BF16)
            nc.gpsimd.dma_start(out=wq, in_=wq_d)
            wk = consts.tile([2 * R, C], BF16)
            nc.gpsimd.dma_start(out=wk, in_=wk_d)
            wv = consts.tile([2 * R, C], BF16)
            nc.gpsimd.dma_start(out=wv, in_=wv_d)
            maskr = consts.tile([128, 1024], BF16)
            nc.gpsimd.dma_start(out=maskr, in_=maskr_d)
            ident = consts.tile([128, 128], F32)
            nc.gpsimd.dma_start(out=ident, in_=ident_d)
            identb = consts.tile([128, 128], BF16)
            nc.gpsimd.dma_start(out=identb, in_=identb_d)
            cvc = consts.tile([128, 4], F32)
            nc.gpsimd.dma_start(out=cvc, in_=cvc_d)
            dwc = consts.tile([128, 4], F32)
            nc.gpsimd.dma_start(out=dwc, in_=dwc_d)
            xres = consts.tile([128, 4, RPC], F32)
            pwtb = consts.tile([128, 4, C], BF16)
            pw1t = consts.tile([128, 4, R], F32)
            pw1tb = consts.tile([128, 4, R], BF16)
            pw2t = consts.tile([R, C], F32)
            pw2tb = consts.tile([R, C], BF16)

            # persistent intermediates
            xrt = persist.tile([R, RPC], BF16)             # xr^T (core rows)
            xkvt = persist.tile([2 * R, N], BF16)          # xkv^T (all rows)
            qt2 = persist.tile([128, 4, RPC], BF16)        # Q^T head pairs
            kt2 = persist.tile([128, 4, N], BF16)          # K^T head pairs
            v2 = persist.tile([128, IC, 4, 128], BF16)     # V rows, head pairs
            et = persist.tile([128, 4096], BF16)           # masked diag-block exp
            dsums = persist.tile([128, H * IC * 3], F32)   # exp row-sum parts
            dsums0 = persist.tile([128, 5], F32)           # first group: 4 parts + total
            dtot = persist.tile([128, H * IC * 2], F32)
            recips = persist.tile([128, H * IC], F32)
            wvt = persist.tile([128, 4, RPC], F32)         # wv^T assembled [c, i]
            dyn0b = persist.tile([128, 4, RPC], BF16)      # (wv*dw)^T bf16
            ya = persist.tile([128, 4, RPC], F32)          # fp32 x-path output
            y1a = persist.tile([128, 4, RPC], F32)
            y1b = persist.tile([128, 4, RPC], BF16)
            t2a = persist.tile([R, RPC], F32)
            t2b = persist.tile([R, RPC], BF16)
            scratch = persist.tile([128, 1536], FP8)       # ACT exp discard target

            def sps(shape):
                return psS.tile(shape, F32, tag="s", name="s_tile")

            def exp_part0_with_diag(psq, ln, acc, p, hh, ic):
                """Exp the part-0 score slab (j < 1536 includes the core's own
                rows), keeping the output in rotating bf16 scratch so the
                (h, ic) diagonal block can be extracted by PE transpose --
                saves the dedicated diag exps on the bottleneck ACT engine."""
                sc = work.tile([128, 1536], F32, tag="sc0", name="sc0")
                ai = nc.scalar.activation(sc[:, 0:ln], psq[:, 0:ln], AF.Exp,
                                          accum_out=acc)
                tblk = (p * 8 + hh * IC + ic) * 128
                pstd = sps([128, 128])
                nc.tensor.transpose(pstd, sc[:, ic * 128:(ic + 1) * 128],
                                    ident)
                nc.vector.tensor_mul(et[:, tblk:tblk + 128], pstd,
                                     maskr[:, 0:128])
                return ai

            def project_pair(p, jcs=range(8), head=True):
                """Q^T/K^T/V for heads (2p, 2p+1), stacked on partitions."""
                psl = slice(p * 128, (p + 1) * 128)
                if head:
                    ps = sps([128, 512])
                    nc.tensor.matmul(ps, wq[:, psl], xrt, start=True, stop=True)
                    nc.vector.tensor_copy(qt2[:, p, :], ps)
                for jc in jcs:
                    js = slice(jc * 512, (jc + 1) * 512)
                    ps = sps([128, 512])
                    nc.tensor.matmul(ps, wk[:, psl], xkvt[:, js],
                                     start=True, stop=True)
                    nc.vector.tensor_copy(kt2[:, p, js], ps)
                if head:
                    for ic in range(IC):
                        cs = slice(ic * 128, (ic + 1) * 128)
                        ps = sps([128, 128])
                        nc.tensor.matmul(ps, xkvt[:, cs], wv[:, psl],
                                         start=True, stop=True)
                        nc.vector.tensor_copy(v2[:, ic, p, :], ps)

            anchors = {}

            def den_num_pair(p):
                """Denominator exp+rowsum stream and per-(h,ic) numerators."""
                for hh in range(2):
                    h = 2 * p + hh
                    poff = hh * 64
                    for ic in range(IC):
                        if p == 0 and hh == 0 and ic == 0:
                            continue   # handled in the interleaved prologue
                        cs = slice(ic * 128, (ic + 1) * 128)
                        ki = h * IC + ic
                        k0 = ki * 3
                        for kpart, (off, ln) in enumerate(DEN_PARTS):
                            psq = psDen.tile([128, 1536], F32, tag="big")
                            for m in range(ln // 512):
                                nc.tensor.matmul(
                                    psq[:, m * 512:(m + 1) * 512],
                                    qt2[poff:poff + 64, p, cs],
                                    kt2[poff:poff + 64, p,
                                        off + m * 512:off + (m + 1) * 512],
                                    start=True, stop=True)
                            acc = dsums[:, k0 + kpart:k0 + kpart + 1]
                            if kpart == 0:
                                ai = exp_part0_with_diag(psq, ln, acc,
                                                         p, hh, ic)
                            else:
                                ai = nc.scalar.activation(
                                    scratch[:, 0:ln], psq[:, 0:ln], AF.Exp,
                                    accum_out=acc)
                            anchors.setdefault(p, []).append(ai)
                        nc.vector.tensor_add(dtot[:, 2 * ki:2 * ki + 1],
                                             dsums[:, k0:k0 + 1],
                                             dsums[:, k0 + 1:k0 + 2])
                        nc.vector.tensor_add(dtot[:, 2 * ki + 1:2 * ki + 2],
                                             dtot[:, 2 * ki:2 * ki + 1],
                                             dsums[:, k0 + 2:k0 + 3])
                        nc.vector.reciprocal(recips[:, ki:ki + 1],
                                             dtot[:, 2 * ki + 1:2 * ki + 2])
                        # numerator for this (h, ic)
                        tblk = (p * 8 + hh * IC + ic) * 128
                        psw = sps([128, D])
                        nc.tensor.matmul(psw, et[:, tblk:tblk + 128],
                                         v2[:, ic, p, poff:poff + 64],
                                         start=True, stop=True)
                        wvs = work.tile([128, D], F32, tag="wvs")
                        nc.vector.tensor_scalar_mul(wvs, psw,
                                                    recips[:, ki:ki + 1])
                        pst = sps([D, 128])
                        nc.tensor.transpose(pst, wvs, ident)
                        nc.vector.tensor_copy(
                            wvt[poff:poff + 64, p, ic * 128:(ic + 1) * 128], pst)
                # attention-path depthwise scale (dw_b folded into cvec)
                nc.vector.tensor_scalar_mul(wvt[:, p, :], wvt[:, p, :],
                                            dwc[:, p:p + 1])
                nc.vector.tensor_copy(dyn0b[:, p, :], wvt[:, p, :])

            def late_loads():
                """Epilogue-only tensors: DMA them only after pair-0's first
                exps so they don't steal DMA-bus bandwidth from xt."""
                from concourse.bass import _add_dep_helper
                a = anchors[0][2].ins
                for dmi in (
                    nc.gpsimd.dma_start(
                        out=xres,
                        in_=xres_d.rearrange("(c p) j -> p c j", p=128)),
                    nc.gpsimd.dma_start(
                        out=pwtb,
                        in_=pwtb_d.rearrange("(c p) r -> p c r", p=128)),
                    nc.gpsimd.dma_start(
                        out=pw1t,
                        in_=pw1t_d.rearrange("(c p) r -> p c r", p=128)),
                    nc.gpsimd.dma_start(
                        out=pw1tb,
                        in_=pw1tb_d.rearrange("(c p) r -> p c r", p=128)),
                    nc.gpsimd.dma_start(out=pw2t, in_=pw2t_d),
                    nc.gpsimd.dma_start(out=pw2tb, in_=pw2tb_d),
                ):
                    _add_dep_helper(dmi.ins, a, True, "defer epilogue DMAs")

            def xpath():
                """fp32 x-path: ya = (x + cvec) @ p1^T @ p2^T (runs under the
                ACT shadow mid-stream)."""
                from concourse.bass import _add_dep_helper
                for t in range(4):
                    ai = nc.vector.tensor_scalar_add(y1a[:, t, :],
                                                     xres[:, t, :],
                                                     cvc[:, t:t + 1])
                    _add_dep_helper(ai.ins, anchors[1][6 * t].ins, False,
                                    "spread xpath adds across pair 1")
                psa = sps([R, 512])
                for c in range(4):
                    mi = nc.tensor.matmul(psa, pw1t[:, c, :], y1a[:, c, :],
                                          start=(c == 0), stop=(c == 3))
                    _add_dep_helper(mi.ins, anchors[2][2 + 4 * c].ins, False,
                                    "spread xpath p1 across pair 2")
                nc.vector.tensor_copy(t2a, psa)
                for t in range(4):
                    pya = sps([128, 512])
                    mi = nc.tensor.matmul(pya, pw2t[:, t * 128:(t + 1) * 128],
                                          t2a, start=True, stop=True)
                    _add_dep_helper(mi.ins, anchors[2][18 + t].ins, False,
                                    "spread xpath p2 across pair 2")
                    nc.vector.tensor_copy(ya[:, t, :], pya)

            def xkvt_chunk(jc):
                js = slice(jc * 512, (jc + 1) * 512)
                ps = sps([2 * R, 512])
                for c in range(4):
                    nc.tensor.matmul(ps, kvw1t[:, c, :], xt[:, c, js],
                                     start=(c == 0), stop=(c == 3))
                nc.vector.tensor_copy(xkvt[:, js], ps)

            # ---- startup: emit only what the first score scans need, so
            # PE's in-order stream is not stuck behind late xt slabs ----
            ps = sps([R, 512])
            for c in range(4):
                nc.tensor.matmul(ps, qw1t[:, c, :], xt[:, c, 0:RPC],
                                 start=(c == 0), stop=(c == 3))
            nc.vector.tensor_copy(xrt, ps)
            for jc in range(3):
                xkvt_chunk(jc)
            project_pair(0, jcs=range(3))

            # first (h0, ic0) group: parts interleaved with the remaining
            # xkvt/kt chunk projections in j order
            poff0 = 0
            cs0 = slice(0, 128)
            parts0 = [(0, 512), (512, 1024), (1536, 1536), (3072, 1024)]
            for kpart, (off, ln) in enumerate(parts0):
                psq = psDen.tile([128, 1536], F32, tag="big")
                for m in range(ln // 512):
                    nc.tensor.matmul(
                        psq[:, m * 512:(m + 1) * 512],
                        qt2[poff0:poff0 + 64, 0, cs0],
                        kt2[poff0:poff0 + 64, 0,
                            off + m * 512:off + (m + 1) * 512],
                        start=True, stop=True)
                if kpart == 0:
                    ai = exp_part0_with_diag(psq, ln, dsums0[:, 0:1], 0, 0, 0)
                else:
                    ai = nc.scalar.activation(
                        scratch[:, 0:ln], psq[:, 0:ln], AF.Exp,
                        accum_out=dsums0[:, kpart:kpart + 1])
                anchors.setdefault(0, []).append(ai)
                if kpart == 0:
                    xkvt_chunk(3)
                    project_pair(0, jcs=[3], head=False)
                elif kpart == 1:
                    for jc in (4, 5):
                        xkvt_chunk(jc)
                    project_pair(0, jcs=[4, 5], head=False)
                elif kpart == 2:
                    for jc in (6, 7):
                        xkvt_chunk(jc)
                    project_pair(0, jcs=[6, 7], head=False)
            nc.vector.tensor_add(dtot[:, 0:1], dsums0[:, 0:1], dsums0[:, 1:2])
            nc.vector.tensor_add(dtot[:, 1:2], dsums0[:, 2:3], dsums0[:, 3:4])
            nc.vector.tensor_add(dsums0[:, 4:5], dtot[:, 0:1], dtot[:, 1:2])
            nc.vector.reciprocal(recips[:, 0:1], dsums0[:, 4:5])
            psw = sps([128, D])
            nc.tensor.matmul(psw, et[:, 0:128], v2[:, 0, 0, 0:64],
                             start=True, stop=True)
            wvs = work.tile([128, D], F32, tag="wvs")
            nc.vector.tensor_scalar_mul(wvs, psw, recips[:, 0:1])
            pst = sps([D, 128])
            nc.tensor.transpose(pst, wvs, ident)
            nc.vector.tensor_copy(wvt[0:64, 0, 0:128], pst)

            for p in range(4):
                if p < 4 - 1:
                    project_pair(p + 1)   # executes under pair p's ACT shadow
                den_num_pair(p)
                if p == 0:
                    late_loads()
                if p == 2:
                    xpath()   # fp32 x-path, runs under the ACT shadow

            # ---- bf16 attention-path epilogue + combine ----
            for tp in range(4):
                pse = psDen.tile([128, 1536], F32, tag="big", name="pse")
                for c in range(4):
                    nc.tensor.matmul(pse[:, 0:512],
                                     pwtb[:, c, tp * 128:(tp + 1) * 128],
                                     dyn0b[:, c, :], start=(c == 0), stop=(c == 3))
                nc.vector.tensor_copy(y1b[:, tp, :], pse[:, 0:512])
            psb = sps([R, 512])
            for c in range(4):
                nc.tensor.matmul(psb, pw1tb[:, c, :], y1b[:, c, :],
                                 start=(c == 0), stop=(c == 3))
            nc.vector.tensor_copy(t2b, psb)
            for t in range(4):
                psy = psDen.tile([128, 1536], F32, tag="big", name="psy")
                nc.tensor.matmul(psy[:, 0:512], pw2tb[:, t * 128:(t + 1) * 128],
                                 t2b, start=True, stop=True)
                ysb = work.tile([128, RPC], F32, tag="ysb")
                nc.vector.tensor_add(ysb, psy[:, 0:512], ya[:, t, :])
                eng = nc.sync if t % 2 == 0 else nc.gpsimd
                eng.dma_start(out=yt_d[t * 128:(t + 1) * 128, :], in_=ysb)

    nc.compile()
    return nc


def _prep_inputs(inputs):
    x = np.asarray(inputs["x"], np.float32)[0]        # [N, C]
    q_w1 = np.asarray(inputs["q_w1"], np.float32)
    q_w2 = np.asarray(inputs["q_w2"], np.float32)
    kv_w1 = np.asarray(inputs["kv_w1"], np.float32)
    kv_w2 = np.asarray(inputs["kv_w2"], np.float32)
    dw_w = np.asarray(inputs["dw_w"], np.float32)
    dw_b = np.asarray(inputs["dw_b"], np.float32)
    pw_w = np.asarray(inputs["pw_w"], np.float32)
    pw_b = np.asarray(inputs["pw_b"], np.float32)
    p_w1 = np.asarray(inputs["p_w1"], np.float32)
    p_w2 = np.asarray(inputs["p_w2"], np.float32)

    xT = np.ascontiguousarray(x.T)                    # [C, N]
    xT_bf = xT.astype(bf16)

    wq = np.empty((R, C), np.float32)
    wkm = np.empty((2 * R, C), np.float32)
    wvm = np.empty((2 * R, C), np.float32)
    for h in range(H):
        hs = slice(h * D, (h + 1) * D)
        wq[:, hs] = q_w2[hs, :].T * SCALE
        wkm[:, hs] = kv_w2[hs, :].T
        wvm[:, hs] = kv_w2[C + h * D:C + (h + 1) * D, :].T

    jj, ii = np.meshgrid(np.arange(128), np.arange(128), indexing="ij")
    maskt = (((ii // BLK) == (jj // BLK)) & (ii >= jj)).astype(bf16)
    maskr = np.tile(maskt, (1, 8))
    cvec = dw_b @ pw_w.T + pw_b

    shared = {
        "qw1t": np.ascontiguousarray(q_w1.T).astype(bf16),
        "kvw1t": np.ascontiguousarray(kv_w1.T).astype(bf16),
        "wq": wq.astype(bf16),
        "wk": wkm.astype(bf16),
        "wv": wvm.astype(bf16),
        "pwtb": np.ascontiguousarray(pw_w.T).astype(bf16),
        "pw1t": np.ascontiguousarray(p_w1.T),
        "pw2t": np.ascontiguousarray(p_w2.T),
        "pw1tb": np.ascontiguousarray(p_w1.T).astype(bf16),
        "pw2tb": np.ascontiguousarray(p_w2.T).astype(bf16),
        "dwc": np.ascontiguousarray(dw_w.reshape(4, 128).T),
        "cvc": np.ascontiguousarray(cvec.reshape(4, 128).T),
        "maskr": np.ascontiguousarray(maskr),
        "ident": np.eye(128, dtype=np.float32),
        "identb": np.eye(128, dtype=np.float32).astype(bf16),
    }
    in_maps = []
    for core in range(NCORES):
        r0 = core * RPC
        rolled = np.concatenate([xT_bf[:, r0:], xT_bf[:, :r0]], axis=1)
        m = dict(shared)
        m["xt"] = np.ascontiguousarray(rolled)
        m["xres"] = np.ascontiguousarray(xT[:, r0:r0 + RPC])
        in_maps.append(m)
    return in_maps


def kernel(**inputs):
    if "nc" not in _CACHE:
        _CACHE["nc"] = _build_program()
    nc = _CACHE["nc"]
    in_maps = _prep_inputs(inputs)
    res = run_bass_kernel_spmd(nc, in_maps, core_ids=list(range(NCORES)))
    y = np.empty((N, C), np.float32)
    for core in range(NCORES):
        r0 = core * RPC
        y[r0:r0 + RPC, :] = res.results[core]["yt"].T
    return y.reshape(1, N, C)
